# revision 1
# baseline (speedup 1.0000x reference)
"""Trainium2 Bass kernel for nn_BasicBlockA (PixelCNN-style masked-conv block).

Math (see reference):
  w1 = (weight1*mask0 + softplus(center1)*mask1) * mask      [16,3,3,3,3]
  h  = elu(conv2d(x, w1.reshape(48,3,3,3), pad=1) + bias1)   [B,48,H,W]
  h2 = grouped_conv(h, w2.reshape(48,3,3,3), groups=16)      [B,48,H,W]
  out = h2.reshape(B,16,3,H,W).mean(1) + res*(res>0)*x

Device strategy (pure data parallel, 8 images per core, raw Block/semaphore
style -- this walrus build rejects the multi-wait sync_info TileContext
emits; standalone wait_ge instructions work):
  - grouped conv + mean folds into a single 48->3 conv with weights/16.
  - all matmuls in float32r (1 cycle/row at N=512 vs 4 for fp32).
  - stage 1: host ships a (dy,ci)-stacked padded image [9(+ones),128,130];
    each 512-px block is 3 accumulating K=9/10 matmuls (dx shifts are
    free-dim AP offsets; bias rides the ones-row on the center dx).
  - exact ELU across three engines (every RAW edge cross-engine):
    ACT e=Exp(psum); GPSIMD t=min(e,1)-1; DVE h1=max(psum, t), written
    twice: f32r at partitions 0-47 and bf16 at partitions 64-111.
  - stage 2: 9 taps split 5/4 over two concurrent PE row groups
    (tile_position (0,0) f32r incl. residual-identity center tap with K=51,
    and (64,0) bf16 -- f32r weights crash HW at non-zero row base), all
    accumulating into one PSUM bank; free-dim AP shifts for dy/dx.
  - per-image pipeline: 4-deep PSUM slots, double-buffered ELU staging and
    output chunks; one DMA semaphore per purpose (concurrent DMA
    completions are unordered, cumulative thresholds on a shared sem race).
"""

import numpy as np

PERCORE = 8
N_CORES = 8
C, L, KK = 3, 16, 3
H = W = 128
HP = WP = 130
CO1 = L * C  # 48
TAPS = [(dy, dx) for dy in range(3) for dx in range(3)]
NB = 32            # 4-row blocks per image
NIMG = PERCORE

_CACHE = {}


def _softplus(x):
    return np.logaddexp(0.0, x)


def _make_masks(Cc, Kk):
    mid = Kk // 2
    mask0 = np.ones((Cc, Cc, Kk, Kk), np.float32)
    mask1 = np.zeros((Cc, Cc, Kk, Kk), np.float32)
    mask = np.ones((Cc, Cc, Kk, Kk), np.float32)
    for i in range(Cc):
        mask0[i, i, mid, mid] = 0.0
        mask1[i, i, mid, mid] = 1.0
        mask[i, :, mid + 1:, :] = 0.0
        mask[i, :i + 1, mid, mid + 1:] = 0.0
        mask[i, i + 1:, mid, mid:] = 0.0
    return mask0, mask1, mask


def _build_nc():
    import concourse.bass as bass
    import concourse.mybir as mybir

    f32 = mybir.dt.float32
    f32r = mybir.dt.float32r
    bf16 = mybir.dt.bfloat16
    AF = mybir.ActivationFunctionType
    ALU = mybir.AluOpType

    nc = bass.Bass()
    xp_t = nc.declare_dram_parameter("xp", [PERCORE, 3 * C + 1, H, WP], f32r, False)
    w1_t = nc.declare_dram_parameter("w1", [3 * C + 1, 3, CO1], f32r, False)
    w2_t = nc.declare_dram_parameter("w2", [CO1 + 3, 9, 3], f32r, False)
    w2b_t = nc.declare_dram_parameter("w2b", [CO1, 9, 3], bf16, False)
    out_t = nc.declare_dram_parameter("out", [PERCORE, 3, H, W], f32, True)

    from contextlib import ExitStack
    with ExitStack() as ctx:
        w1sb = ctx.enter_context(nc.sbuf_tensor([128, 3, CO1], f32r))
        w2sb = ctx.enter_context(nc.sbuf_tensor([128, 9, 3], f32r))
        xp_sb = ctx.enter_context(nc.sbuf_tensor([128, H, WP], f32r))
        h1 = ctx.enter_context(nc.sbuf_tensor([128, HP, WP], f32r))
        h1r = ctx.enter_context(nc.sbuf_tensor([128, HP, WP], bf16))
        w2b_sb = ctx.enter_context(nc.sbuf_tensor([128, 9, 3], bf16))
        e_sb = ctx.enter_context(nc.sbuf_tensor([CO1, 2, 4, 128], f32))
        tt_sb = ctx.enter_context(nc.sbuf_tensor([CO1, 2, 4, 128], f32))
        out_sb = ctx.enter_context(nc.sbuf_tensor([3, 2, 32, 128], f32))
        ps1 = ctx.enter_context(nc.psum_tensor([CO1, 4, 4, 128], f32))
        ps2 = ctx.enter_context(nc.psum_tensor([3, 4, 4, 128], f32))
        wdma = ctx.enter_context(nc.semaphore("wdma"))
        xdma = ctx.enter_context(nc.semaphore("xdma"))
        rdma = ctx.enter_context(nc.semaphore("rdma"))
        mset = ctx.enter_context(nc.semaphore("mset"))
        s1pe = ctx.enter_context(nc.semaphore("s1pe"))
        acts = ctx.enter_context(nc.semaphore("acts"))
        tsg = ctx.enter_context(nc.semaphore("tsg"))
        elu = ctx.enter_context(nc.semaphore("elu"))
        s2pe = ctx.enter_context(nc.semaphore("s2pe"))
        cp = ctx.enter_context(nc.semaphore("cp"))
        odma0 = ctx.enter_context(nc.semaphore("odma0"))
        odma1 = ctx.enter_context(nc.semaphore("odma1"))
        block = ctx.enter_context(nc.Block())

        @block.gpsimd
        def _(gpsimd):
            nc.gpsimd.memset(h1[0:CO1 + 3, :, :].bitcast(f32), 0.0)
            nc.gpsimd.memset(h1r[64:64 + CO1, :, :], 0.0).then_inc(mset, 1)
            for i in range(NIMG):
                for b in range(NB):
                    g = NB * i + b
                    gpsimd.wait_ge(acts, g + 1)
                    if g >= 2:
                        gpsimd.wait_ge(elu, g - 1)   # tt slot free
                    nc.gpsimd.tensor_scalar(tt_sb[:, g % 2], e_sb[:, g % 2],
                                            -1.0, 0.0, ALU.add, ALU.min
                                            ).then_inc(tsg, 1)

        @block.sync
        def _(sync):
            # weights once
            sync.dma_start(out=w1sb[0:3 * C + 1, :, :],
                           in_=w1_t[:]).then_inc(wdma, 16)
            sync.dma_start(out=w2sb[0:CO1 + 3, :, :],
                           in_=w2_t[:]).then_inc(wdma, 16)
            sync.dma_start(out=w2b_sb[64:64 + CO1, :, :],
                           in_=w2b_t[:]).then_inc(wdma, 16)
            # per image: xp in, residual rows in, outputs of previous image
            for i in range(NIMG):
                if i > 0:
                    sync.wait_ge(s1pe, NB * i)        # xp_sb free
                sync.dma_start(out=xp_sb[0:3 * C + 1, :, :],
                               in_=xp_t[i]).then_inc(xdma, 16)
                if i > 0:
                    # out-DMAs of image i-1 must be issued BEFORE blocking on
                    # s2pe: stage-2 progress depends on them via out_sb slots
                    for c in range(4):
                        sync.wait_ge(cp, NB * (i - 1) + 8 * (c + 1))
                        sync.dma_start(
                            out=out_t[i - 1, :, 32 * c:32 * c + 32, :],
                            in_=out_sb[:, (4 * (i - 1) + c) % 2]).then_inc(
                                odma0 if (4 * (i - 1) + c) % 2 == 0 else odma1, 16)
                if i == 0:
                    sync.wait_ge(mset, 1)
                else:
                    sync.wait_ge(s2pe, NB * i)        # h1 resid rows free
                sync.dma_start(out=h1[CO1:CO1 + 3, 1:129, :],
                               in_=xp_t[i, C:2 * C, :, :]).then_inc(rdma, 16)
            for c in range(4):
                sync.wait_ge(cp, NB * (NIMG - 1) + 8 * (c + 1))
                sync.dma_start(out=out_t[NIMG - 1, :, 32 * c:32 * c + 32, :],
                               in_=out_sb[:, (4 * (NIMG - 1) + c) % 2]
                               ).then_inc(
                                   odma0 if (4 * (NIMG - 1) + c) % 2 == 0
                                   else odma1, 16)

        @block.tensor
        def _(tensor):
            for i in range(NIMG):
                for b in range(NB):  # stage 1
                    g = NB * i + b
                    if g >= 4:
                        tensor.wait_ge(elu, g - 3)          # ps1 slot free
                    if b == 0:
                        if i == 0:
                            tensor.wait_ge(wdma, 48)
                        tensor.wait_ge(xdma, 16 * (i + 1))
                    ps = ps1[:, g % 4]
                    for dx in range(3):
                        kk = 3 * C + 1 if dx == 1 else 3 * C
                        mm = nc.tensor.matmul(
                            ps,
                            w1sb[0:kk, dx, :],
                            xp_sb[0:kk, 4 * b:4 * b + 4, dx:dx + 128],
                            start=(dx == 0), stop=(dx == 2))
                        if dx == 2:
                            mm.then_inc(s1pe, 1)
                for b in range(NB):  # stage 2
                    g = NB * i + b
                    if g >= 4:
                        tensor.wait_ge(cp, g - 3)           # ps2 slot free
                    tensor.wait_ge(elu, NB * i + min(NB, b + 2))
                    if b == 0:
                        tensor.wait_ge(rdma, 16 * (i + 1))  # resid rows
                    ps = ps2[:, g % 4]
                    order = [(0, 0), (5, 1), (1, 0), (6, 1), (2, 0), (7, 1),
                             (3, 0), (8, 1), (4, 0)]
                    for idx, (t, grp) in enumerate(order):
                        dy, dx = divmod(t, 3)
                        first, last = idx == 0, idx == len(order) - 1
                        if grp == 0:
                            kk = CO1 + 3 if t == 4 else CO1
                            mm = nc.tensor.matmul(
                                ps,
                                w2sb[0:kk, t, :],
                                h1[0:kk, 4 * b + dy:4 * b + dy + 4,
                                   dx:dx + 128],
                                start=first, stop=last)
                        else:
                            mm = nc.tensor.matmul(
                                ps,
                                w2b_sb[64:64 + CO1, t, :],
                                h1r[64:64 + CO1, 4 * b + dy:4 * b + dy + 4,
                                    dx:dx + 128],
                                start=first, stop=last,
                                tile_position=(64, 0))
                        if last:
                            mm.then_inc(s2pe, 1)

        @block.scalar
        def _(scalar):
            for i in range(NIMG):
                for b in range(NB):  # elu exp
                    g = NB * i + b
                    scalar.wait_ge(s1pe, g + 1)
                    if g >= 2:
                        scalar.wait_ge(tsg, g - 1)          # e slot free
                    nc.scalar.activation(e_sb[:, g % 2], ps1[:, g % 4], AF.Exp
                                         ).then_inc(acts, 1)
                for b in range(NB):  # stage-2 psum -> out_sb
                    g = NB * i + b
                    scalar.wait_ge(s2pe, g + 1)
                    gc = 4 * i + b // 8
                    if b % 8 == 0 and gc >= 2:
                        scalar.wait_ge(odma0 if gc % 2 == 0 else odma1,
                                       16 * (gc // 2))  # out_sb slot free
                    bb = b % 8
                    nc.scalar.activation(
                        out_sb[:, gc % 2, 4 * bb:4 * bb + 4, :],
                        ps2[:, g % 4], AF.Copy).then_inc(cp, 1)

        @block.vector
        def _(vector):
            for i in range(NIMG):
                for b in range(NB):
                    g = NB * i + b
                    vector.wait_ge(tsg, g + 1)
                    if g == 0:
                        vector.wait_ge(mset, 1)
                    if i > 0:
                        vector.wait_ge(s2pe, NB * (i - 1) + min(NB, b + 2))
                    nc.vector.scalar_tensor_tensor(
                        h1[0:CO1, 4 * b + 1:4 * b + 5, 1:129],
                        tt_sb[:, g % 2], 0.0, ps1[:, g % 4], ALU.add, ALU.max)
                    nc.vector.scalar_tensor_tensor(
                        h1r[64:64 + CO1, 4 * b + 1:4 * b + 5, 1:129],
                        tt_sb[:, g % 2], 0.0, ps1[:, g % 4], ALU.add, ALU.max
                    ).then_inc(elu, 1)

    return nc


def _prep_inputs(x, weight1, center1, bias1, weight2, center2, res):
    mask0, mask1, mask = _make_masks(C, KK)
    w1 = (weight1 * mask0 + _softplus(center1) * mask1) * mask  # [L,C,C,K,K]
    w2 = (weight2 * mask0 + _softplus(center2) * mask1) * mask
    W1 = w1.reshape(CO1, C, KK, KK).astype(np.float32)
    W2m = (w2 / L).transpose(1, 0, 2, 3, 4).reshape(3, CO1, KK, KK)
    W2m = W2m.astype(np.float32)
    rscale = float(res[0] * (res[0] > 0))

    # stage-1 stationary: [(dy,ci)+ones, dx, co]
    w1dev = np.zeros((3 * C + 1, 3, CO1), np.float32)
    w1dev[0:3 * C] = W1.transpose(2, 1, 3, 0).reshape(3 * C, 3, CO1)
    w1dev[3 * C, 1, :] = bias1.reshape(CO1)
    w2dev = np.zeros((CO1 + 3, 9, 3), np.float32)
    w2dev[0:CO1] = W2m.transpose(1, 2, 3, 0).reshape(CO1, 9, 3)
    w2dev[CO1:, 4, :] = rscale * np.eye(3, dtype=np.float32)

    B = x.shape[0]
    xpad = np.zeros((B, C, HP, WP), np.float32)
    xpad[:, :, 1:H + 1, 1:W + 1] = x
    # xpy[(dy,ci), h, w'] = xpad[ci, h+dy, w']; last row = ones (bias)
    xpy = np.empty((B, 3 * C + 1, H, WP), np.float32)
    for dy in range(3):
        xpy[:, 3 * dy:3 * dy + 3] = xpad[:, :, dy:dy + H, :]
    xpy[:, 3 * C] = 1.0
    import ml_dtypes
    w2bdev = w2dev[0:CO1].astype(ml_dtypes.bfloat16)
    return xpy, w1dev, w2dev, w2bdev


def kernel(x, weight1, center1, bias1, weight2, center2, res, _trace=False):
    from concourse.bass_utils import run_bass_kernel_spmd

    xp, w1dev, w2dev, w2bdev = _prep_inputs(
        np.asarray(x, np.float32), np.asarray(weight1, np.float32),
        np.asarray(center1, np.float32), np.asarray(bias1, np.float32),
        np.asarray(weight2, np.float32), np.asarray(center2, np.float32),
        np.asarray(res, np.float32))

    if "nc" not in _CACHE:
        _CACHE["nc"] = _build_nc()
    nc = _CACHE["nc"]

    in_maps = [
        {"xp": xp[i * PERCORE:(i + 1) * PERCORE], "w1": w1dev, "w2": w2dev,
         "w2b": w2bdev}
        for i in range(N_CORES)
    ]
    res_ = run_bass_kernel_spmd(nc, in_maps, list(range(N_CORES)),
                                trace=_trace)
    out = np.concatenate([r["out"] for r in res_.results], axis=0)
    if _trace:
        _CACHE["exec_time_ns"] = res_.exec_time_ns
        _CACHE["profile"] = res_.profile_json
    return out

